# revision 28
# baseline (speedup 1.0000x reference)
"""Trainium2 Bass kernel for a 2-layer modReLU RNN (ExpRNN-style).

Reference computation (per core, batch-sharded 8 ways):
    h0_t = modrelu(x_t @ Wi0.T + bi0 + h0_{t-1} @ Wr0.T, bm0)
    h1_t = modrelu(h0_t @ Wi1.T + bi1 + h1_{t-1} @ Wr1.T, bm1)
    out[b, t] = h1_t ;  final_state = (h0_T, h1_T)

Strategy:
  - Data-parallel over batch: B=64 -> 8 cores x 8 rows. Weights replicated.
  - Phased execution per core:
      G0: U0 = x @ Wi0.T + bi0       (big GEMM over all T, PE-efficient)
      R0: recurrence h0_t = modrelu(U0_t + h0 @ Wr0.T, bm0)   (serial over T)
      G1: U1 = h0 @ Wi1.T + bi1      (big GEMM)
      R1: recurrence h1_t (written out every step)
  - Everything is kept feature-major ("transposed"): activations live as
    h.T with the 1024 feature dim split over 8 x 128 partitions, batch in
    the free dim.  This makes the recurrent matmul layout-stable:
        out[j, b] = sum_k WrT[k, j] * hT[k, b]   (lhsT = Wr.T tile, rhs = hT)
  - modrelu(z, b) = sign(z) * max(|z| + b, 0) computed with 5 DVE ops +
    1 ACT Sign per step, fused over all 1024 features ([128 x 64] tiles).
  - All matmuls fp32 (the recurrence error compounds exponentially; bf16
    was measured at ~5% output error -- unacceptable).
"""

import os
import sys
from contextlib import ExitStack

import numpy as np

sys.path.insert(0, "/opt/trn_rl_repo")
os.environ.setdefault("MYCRO_LOCAL_CACHE", "1")

B, T_FULL, D, H = 64, 512, 1024, 1024
NCORES = 8
BL = B // NCORES  # batch rows per core
KT = 8            # 128-row feature tiles per 1024
JT = 8
P = 128
UPAD = 256        # U tensors padded in bt so the last prefetch reads junk, not OOB

# Tuning knobs (validated defaults; env-overridable for experiments).
COLTILE = int(os.environ.get("RNN_COLTILE", "0"))   # 4x PE col-tiling in R phase
G_F32R = int(os.environ.get("RNN_G_F32R", "0"))     # float32r G-phase matmuls


def _f32():
    from concourse import mybir

    return mybir.dt.float32


def _gemm_phase(ctx, tc, name, W_dram, BI_dram, SRC_dram, UDST_dram, nbt, after=()):
    """U[p, jt, bt] = sum_k W[j, k] * SRC[k-part, kt, bt] + bi[j].

    W_dram: [1024(k), 1024(j)] pre-transposed weight (lhsT source).
    SRC_dram / UDST_dram: [128, 8, nbt(+pad)] feature-major activations.
    """
    import concourse.bass as bass  # noqa: F401
    from concourse import mybir
    from concourse.tile_rust import add_dep_helper

    nc = tc.nc
    f32 = _f32()
    wp = ctx.enter_context(tc.tile_pool(name=f"{name}w", bufs=1))
    cp = ctx.enter_context(tc.tile_pool(name=f"{name}c", bufs=1))
    xp = ctx.enter_context(tc.tile_pool(name=f"{name}x", bufs=2))
    up = ctx.enter_context(tc.tile_pool(name=f"{name}u", bufs=3))
    pp = ctx.enter_context(tc.tile_pool(name=f"{name}p", bufs=2, space="PSUM"))
    writes = []

    W = wp.tile([P, KT * H], f32, tag=f"{name}W")
    for kt in range(KT):
        nc.sync.dma_start(W[:, kt * H:(kt + 1) * H], W_dram[kt * P:(kt + 1) * P, :])
    BI = cp.tile([P, JT], f32, tag=f"{name}BI")
    nc.sync.dma_start(BI[:], BI_dram[:])

    nchunk = nbt // 512
    for c in range(nchunk):
        xs = []
        for kt in range(KT):
            xt_ = xp.tile([P, 512], f32, tag=f"{name}x{kt}")
            rd = nc.sync.dma_start(xt_[:], SRC_dram[:, kt, c * 512:(c + 1) * 512])
            for w in after:
                add_dep_helper(rd.ins, w, reason=f"{name} src read after producer")
            xs.append(xt_)
        for jt in range(JT):
            ps = pp.tile([P, 512], f32, tag=f"{name}ps")
            for kt in range(KT):
                lhsT = W[:, kt * H + jt * P: kt * H + jt * P + P]
                rhs = xs[kt][:]
                if G_F32R:
                    lhsT = lhsT.bitcast(mybir.dt.float32r)
                    rhs = rhs.bitcast(mybir.dt.float32r)
                nc.tensor.matmul(
                    ps[:], lhsT, rhs,
                    start=(kt == 0),
                    stop=(kt == KT - 1),
                )
            u = up.tile([P, 512], f32, tag=f"{name}ut")
            nc.vector.tensor_scalar_add(u[:], ps[:], BI[:, jt:jt + 1])
            w = nc.sync.dma_start(UDST_dram[:, jt, c * 512:(c + 1) * 512], u[:])
            writes.append(w.ins)
    return writes


def _rnn_phase(ctx, tc, name, WR_dram, BMR_dram, U_dram, OUT_dram, nbt,
               fin_dram=None, after=()):
    """Serial recurrence over T steps (nbt = T * BL columns, BL=8).

    h_t.T = modrelu(U[:, :, t*8:(t+1)*8] + Wr.T.T @ h_{t-1}.T, bm)
    Ring buffers hold 16 steps of hT in SBUF; chunks of 16 steps are
    DMA-flushed to OUT_dram (feature-major [128, 8, nbt]).
    """
    import concourse.bass as bass
    from concourse import mybir

    nc = tc.nc
    f32 = _f32()
    ds = bass.ds

    wp = ctx.enter_context(tc.tile_pool(name=f"{name}w", bufs=1))
    cp = ctx.enter_context(tc.tile_pool(name=f"{name}c", bufs=1))
    rp = ctx.enter_context(tc.tile_pool(name=f"{name}r", bufs=1))
    upl = ctx.enter_context(tc.tile_pool(name=f"{name}u", bufs=1))
    ep = ctx.enter_context(tc.tile_pool(name=f"{name}e", bufs=2))
    pp = ctx.enter_context(tc.tile_pool(name=f"{name}p", bufs=2, space="PSUM"))

    W = wp.tile([P, KT * H], f32, tag=f"{name}W")
    for kt in range(KT):
        nc.sync.dma_start(W[:, kt * H:(kt + 1) * H], WR_dram[kt * P:(kt + 1) * P, :])
    BMR = cp.tile([P, 64], f32, tag=f"{name}BMR")
    nc.sync.dma_start(BMR[:], BMR_dram[:])

    ringA = rp.tile([P, 1024], f32, tag=f"{name}rA")
    ringB = rp.tile([P, 1024], f32, tag=f"{name}rB")
    UA = upl.tile([P, 1024], f32, tag=f"{name}uA")
    UB = upl.tile([P, 1024], f32, tag=f"{name}uB")

    def r3(tile_, sl):  # [128, 8, 8] view of ring/U slot sl
        return tile_[:].rearrange("p (k sb) -> p k sb", k=KT)[:, :, sl * BL:(sl + 1) * BL]

    def rflush(tile_):  # [128, 8, 128] view for chunk DMA
        return tile_[:].rearrange("p (k sb) -> p k sb", k=KT)

    from concourse.tile_rust import add_dep_helper

    def dep_on_producers(dma_handle):
        # DRAM-tensor RAW deps across phases are not tracked by Tile; add
        # them explicitly so U reads can't race the producing GEMM's writes.
        for w in after:
            add_dep_helper(dma_handle.ins, w, reason=f"{name} U read after producer")

    # h_{-1} = 0 ; prime UA with the first 16 steps
    flushes = []
    nc.vector.memset(ringB[:], 0.0)
    dep_on_producers(nc.sync.dma_start(rflush(UA), U_dram[:, :, 0:128]))

    with tc.For_i(0, nbt, 256, hint_engines=(mybir.EngineType.PE,)) as i:
        # U for local steps 16..31 of this body
        dep_on_producers(nc.sync.dma_start(rflush(UB), U_dram[:, :, ds(i + 128, 128)]))
        for s in range(32):
            ringW, slotW = (ringA, s) if s < 16 else (ringB, s - 16)
            if s == 0:
                ringR, slotR = ringB, 15
            elif s <= 16:
                ringR, slotR = ringA, s - 1
            else:
                ringR, slotR = ringB, s - 17
            Usrc, slotU = (UA, s) if s < 16 else (UB, s - 16)

            ps = pp.tile([P, 64], f32, tag=f"{name}ps")
            if COLTILE:
                # 4x column tiling: 32-col weight sub-loads go through
                # separate XBUSes and overlap. The whole-bank has_written
                # clear of start=True is unsafe with multiple concurrent
                # groups in one bank, so zero the bank with DVE instead and
                # run every matmul with start=False (accumulate-onto-zero).
                nc.vector.memset(ps[:], 0.0)
                for jt in range(JT):
                    for kt in range(KT):
                        rhs = ringR[:, kt * P + slotR * BL: kt * P + slotR * BL + BL]
                        for c4 in range(4):
                            nc.tensor.matmul(
                                ps[32 * c4:32 * (c4 + 1), jt * BL:(jt + 1) * BL],
                                W[:, kt * H + jt * P + 32 * c4:
                                   kt * H + jt * P + 32 * (c4 + 1)],
                                rhs,
                                start=False, stop=(kt == KT - 1),
                                skip_group_check=True,
                                tile_position=(0, 32 * c4),
                            )
            else:
                for jt in range(JT):
                    for kt in range(KT):
                        nc.tensor.matmul(
                            ps[:, jt * BL:(jt + 1) * BL],
                            W[:, kt * H + jt * P: kt * H + jt * P + P],
                            ringR[:, kt * P + slotR * BL: kt * P + slotR * BL + BL],
                            start=(kt == 0),
                            stop=(kt == KT - 1),
                        )
            # epilogue: h = sign(z) * max(|z| + bm, 0), z = ps + U_t
            z = ep.tile([P, 64], f32, tag=f"{name}z")
            az = ep.tile([P, 64], f32, tag=f"{name}az")
            ab = ep.tile([P, 64], f32, tag=f"{name}ab")
            rr = ep.tile([P, 64], f32, tag=f"{name}rr")
            sg = ep.tile([P, 64], f32, tag=f"{name}sg")
            z3 = z[:].rearrange("p (k b) -> p k b", k=KT)
            nc.vector.tensor_add(z3, ps[:].rearrange("p (k b) -> p k b", k=KT), r3(Usrc, slotU))
            nc.scalar.sign(sg[:], z[:])
            nc.vector.scalar_tensor_tensor(
                az[:], z[:], -1.0, z[:], mybir.AluOpType.mult, mybir.AluOpType.max)
            nc.vector.tensor_add(ab[:], az[:], BMR[:])
            nc.vector.tensor_scalar_max(rr[:], ab[:], 0.0)
            nc.vector.tensor_mul(
                r3(ringW, slotW),
                rr[:].rearrange("p (k b) -> p k b", k=KT),
                sg[:].rearrange("p (k b) -> p k b", k=KT),
            )
            if s == 15:
                flushes.append(nc.sync.dma_start(OUT_dram[:, :, ds(i, 128)], rflush(ringA)).ins)
            elif s == 31:
                flushes.append(nc.sync.dma_start(OUT_dram[:, :, ds(i + 128, 128)], rflush(ringB)).ins)
        # prefetch first half of next body (reads pad junk on the last iter)
        dep_on_producers(nc.sync.dma_start(rflush(UA), U_dram[:, :, ds(i + 256, 128)]))

    if fin_dram is not None:
        nc.sync.dma_start(fin_dram[:], r3(ringB, 15))
    return flushes


def build_nc(T=T_FULL):
    """Build + compile the per-core program. Same SPMD program on all cores."""
    import concourse.tile as tile
    from concourse import bacc

    f32 = _f32()
    nbt = T * BL
    nc = bacc.Bacc("TRN2", target_bir_lowering=False, debug=False, num_devices=NCORES)

    XT = nc.dram_tensor("xt", [P, KT, nbt], f32, kind="ExternalInput").ap()
    WI0 = nc.dram_tensor("wi0t", [D, H], f32, kind="ExternalInput").ap()
    WR0 = nc.dram_tensor("wr0t", [H, H], f32, kind="ExternalInput").ap()
    WI1 = nc.dram_tensor("wi1t", [H, H], f32, kind="ExternalInput").ap()
    WR1 = nc.dram_tensor("wr1t", [H, H], f32, kind="ExternalInput").ap()
    BI0 = nc.dram_tensor("bi0c", [P, JT], f32, kind="ExternalInput").ap()
    BI1 = nc.dram_tensor("bi1c", [P, JT], f32, kind="ExternalInput").ap()
    BM0 = nc.dram_tensor("bm0r", [P, 64], f32, kind="ExternalInput").ap()
    BM1 = nc.dram_tensor("bm1r", [P, 64], f32, kind="ExternalInput").ap()

    OUTT = nc.dram_tensor("outt", [P, KT, nbt + UPAD], f32, kind="ExternalOutput").ap()
    FIN0 = nc.dram_tensor("fin0", [P, 64], f32, kind="ExternalOutput").ap()

    with tile.TileContext(nc) as tc, ExitStack() as top:
        # Internal activations live in a DRAM tile pool so that Tile tracks
        # the cross-phase RAW dependencies (raw dram_tensors are not tracked).
        dp = top.enter_context(tc.tile_pool(name="dram", bufs=1, space="DRAM"))
        U0 = dp.tile([P, KT, nbt + UPAD], f32, tag="u0")
        H0T = dp.tile([P, KT, nbt + UPAD], f32, tag="h0t")
        U1 = dp.tile([P, KT, nbt + UPAD], f32, tag="u1")
        U0, H0T, U1 = U0[:], H0T[:], U1[:]
        # Zero the pad columns: the last loop iteration's U prefetch reads
        # them (by design, as junk) — they must at least be initialized.
        zp = top.enter_context(tc.tile_pool(name="zpad", bufs=1))
        zt = zp.tile([P, KT * 128], f32, tag="zpad")
        nc.vector.memset(zt[:], 0.0)
        z3 = zt[:].rearrange("p (k c) -> p k c", k=KT)
        pad_writes = []
        for U in (U0, U1):
            for o in range(0, UPAD, 128):
                pad_writes.append(
                    nc.sync.dma_start(U[:, :, nbt + o:nbt + o + 128], z3).ins)
        with ExitStack() as ctx:
            w0 = _gemm_phase(ctx, tc, "g0", WI0, BI0, XT, U0, nbt)
        with ExitStack() as ctx:
            fl0 = _rnn_phase(ctx, tc, "r0", WR0, BM0, U0, H0T, nbt,
                             fin_dram=FIN0, after=w0 + pad_writes)
        with ExitStack() as ctx:
            w1 = _gemm_phase(ctx, tc, "g1", WI1, BI1, H0T, U1, nbt, after=fl0)
        with ExitStack() as ctx:
            _rnn_phase(ctx, tc, "r1", WR1, BM1, U1, OUTT, nbt, after=w1 + pad_writes)
    nc.compile()
    return nc


def _prep_inputs(x, Wi0, bi0, Wr0, bm0, Wi1, bi1, Wr1, bm1, T):
    """Host-side shard + transpose. Returns per-core input maps."""

    def col(v):  # [1024] -> [128, 8] feature-tile-major per-partition columns
        return np.ascontiguousarray(v.reshape(JT, P).T)

    def rep(v):  # [1024] -> [128, 64] replicated across the 8 batch columns
        return np.ascontiguousarray(np.repeat(v.reshape(JT, P).T, BL, axis=1))

    shared = {
        "wi0t": np.ascontiguousarray(Wi0.T),
        "wr0t": np.ascontiguousarray(Wr0.T),
        "wi1t": np.ascontiguousarray(Wi1.T),
        "wr1t": np.ascontiguousarray(Wr1.T),
        "bi0c": col(bi0),
        "bi1c": col(bi1),
        "bm0r": rep(bm0),
        "bm1r": rep(bm1),
    }
    in_maps = []
    for c in range(NCORES):
        xc = x[c * BL:(c + 1) * BL, :T, :]  # [8, T, 1024]
        # xt[p, kt, t*8+b] = x[b, t, kt*128+p]
        xt = np.ascontiguousarray(
            xc.reshape(BL, T, KT, P).transpose(3, 2, 1, 0).reshape(P, KT, T * BL)
        )
        in_maps.append({"xt": xt, **shared})
    return in_maps


def _decode_outputs(results, T):
    """results: list of {name: array} per core -> (out [B,T,H], final [2,B,H])."""
    nbt = T * BL
    out = np.empty((B, T, H), np.float32)
    fin = np.empty((2, B, H), np.float32)
    for c in range(NCORES):
        outt = results[c]["outt"][:, :, :nbt]  # [128, 8, nbt]
        # out[b, t, jt*128+p] = outt[p, jt, t*8+b]
        a = outt.reshape(P, JT, T, BL).transpose(3, 2, 1, 0)  # [b, t, jt, p]
        out[c * BL:(c + 1) * BL] = a.reshape(BL, T, H)
        fin[1, c * BL:(c + 1) * BL] = a[:, T - 1].reshape(BL, H)
        f0 = results[c]["fin0"]  # [128, 64] cols = kt*8+b
        fin[0, c * BL:(c + 1) * BL] = f0.reshape(P, KT, BL).transpose(2, 1, 0).reshape(BL, H)
    return out, fin


_NC_CACHE = {}


def _get_nc(T):
    if T not in _NC_CACHE:
        _NC_CACHE[T] = build_nc(T)
    return _NC_CACHE[T]


def run_on_hw(inputs, T=T_FULL, trace=False, tmpdir=None):
    from concourse.bass_utils import run_bass_kernel_spmd

    nc = _get_nc(T)
    in_maps = _prep_inputs(T=T, **inputs)
    res = run_bass_kernel_spmd(
        nc, in_maps, list(range(NCORES)), trace=trace, tmpdir=tmpdir
    )
    out, fin = _decode_outputs(res.results, T)
    return (out, fin), res


def kernel(x, Wi0, bi0, Wr0, bm0, Wi1, bi1, Wr1, bm1):
    inputs = dict(
        x=np.asarray(x, np.float32),
        Wi0=np.asarray(Wi0, np.float32), bi0=np.asarray(bi0, np.float32),
        Wr0=np.asarray(Wr0, np.float32), bm0=np.asarray(bm0, np.float32),
        Wi1=np.asarray(Wi1, np.float32), bi1=np.asarray(bi1, np.float32),
        Wr1=np.asarray(Wr1, np.float32), bm1=np.asarray(bm1, np.float32),
    )
    (out, fin), _ = run_on_hw(inputs, T=T_FULL, trace=False)
    return out, fin


# revision 29
# speedup vs baseline: 1.1138x; 1.1138x over previous
"""Trainium2 Bass kernel for a 2-layer modReLU RNN (ExpRNN-style).

Reference computation (per core, batch-sharded 8 ways):
    h0_t = modrelu(x_t @ Wi0.T + bi0 + h0_{t-1} @ Wr0.T, bm0)
    h1_t = modrelu(h0_t @ Wi1.T + bi1 + h1_{t-1} @ Wr1.T, bm1)
    out[b, t] = h1_t ;  final_state = (h0_T, h1_T)

Strategy:
  - Data-parallel over batch: B=64 -> 8 cores x 8 rows. Weights replicated.
  - Phased execution per core:
      G0: U0 = x @ Wi0.T + bi0       (big GEMM over all T, PE-efficient)
      R0: recurrence h0_t = modrelu(U0_t + h0 @ Wr0.T, bm0)   (serial over T)
      G1: U1 = h0 @ Wi1.T + bi1      (big GEMM)
      R1: recurrence h1_t (written out every step)
  - Everything is kept feature-major ("transposed"): activations live as
    h.T with the 1024 feature dim split over 8 x 128 partitions, batch in
    the free dim.  This makes the recurrent matmul layout-stable:
        out[j, b] = sum_k WrT[k, j] * hT[k, b]   (lhsT = Wr.T tile, rhs = hT)
  - modrelu(z, b) = sign(z) * max(|z| + b, 0) computed with 5 DVE ops +
    1 ACT Sign per step, fused over all 1024 features ([128 x 64] tiles).
  - All matmuls fp32 (the recurrence error compounds exponentially; bf16
    was measured at ~5% output error -- unacceptable).
"""

import os
import sys
from contextlib import ExitStack

import numpy as np

sys.path.insert(0, "/opt/trn_rl_repo")
os.environ.setdefault("MYCRO_LOCAL_CACHE", "1")

B, T_FULL, D, H = 64, 512, 1024, 1024
NCORES = 8
BL = B // NCORES  # batch rows per core
KT = 8            # 128-row feature tiles per 1024
JT = 8
P = 128
UPAD = 256        # U tensors padded in bt so the last prefetch reads junk, not OOB

# Tuning knobs (validated defaults; env-overridable for experiments).
COLTILE = int(os.environ.get("RNN_COLTILE", "0"))   # 4x PE col-tiling in R phase
G_F32R = int(os.environ.get("RNN_G_F32R", "0"))     # float32r G-phase matmuls


def _f32():
    from concourse import mybir

    return mybir.dt.float32


def _gemm_phase(ctx, tc, name, W_dram, BI_dram, SRC_dram, UDST_dram, nbt, after=()):
    """U[p, jt, bt] = sum_k W[j, k] * SRC[k-part, kt, bt] + bi[j].

    W_dram: [1024(k), 1024(j)] pre-transposed weight (lhsT source).
    SRC_dram / UDST_dram: [128, 8, nbt(+pad)] feature-major activations.
    """
    import concourse.bass as bass  # noqa: F401
    from concourse import mybir
    from concourse.tile_rust import add_dep_helper

    nc = tc.nc
    f32 = _f32()
    wp = ctx.enter_context(tc.tile_pool(name=f"{name}w", bufs=1))
    cp = ctx.enter_context(tc.tile_pool(name=f"{name}c", bufs=1))
    xp = ctx.enter_context(tc.tile_pool(name=f"{name}x", bufs=2))
    up = ctx.enter_context(tc.tile_pool(name=f"{name}u", bufs=3))
    pp = ctx.enter_context(tc.tile_pool(name=f"{name}p", bufs=2, space="PSUM"))
    writes = []

    W = wp.tile([P, KT * H], f32, tag=f"{name}W")
    for kt in range(KT):
        nc.sync.dma_start(W[:, kt * H:(kt + 1) * H], W_dram[kt * P:(kt + 1) * P, :])
    BI = cp.tile([P, JT], f32, tag=f"{name}BI")
    nc.sync.dma_start(BI[:], BI_dram[:])

    nchunk = nbt // 512
    for c in range(nchunk):
        xs = []
        for kt in range(KT):
            xt_ = xp.tile([P, 512], f32, tag=f"{name}x{kt}")
            rd = nc.sync.dma_start(xt_[:], SRC_dram[:, kt, c * 512:(c + 1) * 512])
            for w in after:
                add_dep_helper(rd.ins, w, reason=f"{name} src read after producer")
            xs.append(xt_)
        for jt in range(JT):
            ps = pp.tile([P, 512], f32, tag=f"{name}ps")
            for kt in range(KT):
                lhsT = W[:, kt * H + jt * P: kt * H + jt * P + P]
                rhs = xs[kt][:]
                if G_F32R:
                    lhsT = lhsT.bitcast(mybir.dt.float32r)
                    rhs = rhs.bitcast(mybir.dt.float32r)
                nc.tensor.matmul(
                    ps[:], lhsT, rhs,
                    start=(kt == 0),
                    stop=(kt == KT - 1),
                )
            u = up.tile([P, 512], f32, tag=f"{name}ut")
            nc.vector.tensor_scalar_add(u[:], ps[:], BI[:, jt:jt + 1])
            w = nc.sync.dma_start(UDST_dram[:, jt, c * 512:(c + 1) * 512], u[:])
            writes.append(w.ins)
    return writes


def _rnn_phase(ctx, tc, name, WR_dram, BMR_dram, U_dram, OUT_dram, nbt,
               fin_dram=None, after=()):
    """Serial recurrence over T steps (nbt = T * BL columns, BL=8).

    h_t.T = modrelu(U[:, :, t*8:(t+1)*8] + Wr.T.T @ h_{t-1}.T, bm)
    Ring buffers hold 16 steps of hT in SBUF; chunks of 16 steps are
    DMA-flushed to OUT_dram (feature-major [128, 8, nbt]).
    """
    import concourse.bass as bass
    from concourse import mybir

    nc = tc.nc
    f32 = _f32()
    ds = bass.ds

    wp = ctx.enter_context(tc.tile_pool(name=f"{name}w", bufs=1))
    cp = ctx.enter_context(tc.tile_pool(name=f"{name}c", bufs=1))
    rp = ctx.enter_context(tc.tile_pool(name=f"{name}r", bufs=1))
    upl = ctx.enter_context(tc.tile_pool(name=f"{name}u", bufs=1))
    ep = ctx.enter_context(tc.tile_pool(name=f"{name}e", bufs=2))
    pp = ctx.enter_context(tc.tile_pool(name=f"{name}p", bufs=2, space="PSUM"))

    W = wp.tile([P, KT * H], f32, tag=f"{name}W")
    for kt in range(KT):
        nc.sync.dma_start(W[:, kt * H:(kt + 1) * H], WR_dram[kt * P:(kt + 1) * P, :])
    BMR = cp.tile([P, 64], f32, tag=f"{name}BMR")
    nc.sync.dma_start(BMR[:], BMR_dram[:])

    ringA = rp.tile([P, 1024], f32, tag=f"{name}rA")
    ringB = rp.tile([P, 1024], f32, tag=f"{name}rB")
    UA = upl.tile([P, 1024], f32, tag=f"{name}uA")
    UB = upl.tile([P, 1024], f32, tag=f"{name}uB")

    def r3(tile_, sl):  # [128, 8, 8] view of ring/U slot sl
        return tile_[:].rearrange("p (k sb) -> p k sb", k=KT)[:, :, sl * BL:(sl + 1) * BL]

    def rflush(tile_):  # [128, 8, 128] view for chunk DMA
        return tile_[:].rearrange("p (k sb) -> p k sb", k=KT)

    from concourse.tile_rust import add_dep_helper

    def dep_on_producers(dma_handle):
        # DRAM-tensor RAW deps across phases are not tracked by Tile; add
        # them explicitly so U reads can't race the producing GEMM's writes.
        for w in after:
            add_dep_helper(dma_handle.ins, w, reason=f"{name} U read after producer")

    # h_{-1} = 0 ; prime UA with the first 16 steps
    flushes = []
    nc.vector.memset(ringB[:], 0.0)
    dep_on_producers(nc.sync.dma_start(rflush(UA), U_dram[:, :, 0:128]))

    with tc.For_i(0, nbt, 256, hint_engines=(mybir.EngineType.PE,)) as i:
        # U for local steps 16..31 of this body
        dep_on_producers(nc.sync.dma_start(rflush(UB), U_dram[:, :, ds(i + 128, 128)]))
        for s in range(32):
            ringW, slotW = (ringA, s) if s < 16 else (ringB, s - 16)
            if s == 0:
                ringR, slotR = ringB, 15
            elif s <= 16:
                ringR, slotR = ringA, s - 1
            else:
                ringR, slotR = ringB, s - 17
            Usrc, slotU = (UA, s) if s < 16 else (UB, s - 16)

            ps = pp.tile([P, 64], f32, tag=f"{name}ps")
            if COLTILE:
                # 4x column tiling: 32-col weight sub-loads go through
                # separate XBUSes and overlap. The whole-bank has_written
                # clear of start=True is unsafe with multiple concurrent
                # groups in one bank, so zero the bank with DVE instead and
                # run every matmul with start=False (accumulate-onto-zero).
                nc.vector.memset(ps[:], 0.0)
                for jt in range(JT):
                    for kt in range(KT):
                        rhs = ringR[:, kt * P + slotR * BL: kt * P + slotR * BL + BL]
                        for c4 in range(4):
                            nc.tensor.matmul(
                                ps[32 * c4:32 * (c4 + 1), jt * BL:(jt + 1) * BL],
                                W[:, kt * H + jt * P + 32 * c4:
                                   kt * H + jt * P + 32 * (c4 + 1)],
                                rhs,
                                start=False, stop=(kt == KT - 1),
                                skip_group_check=True,
                                tile_position=(0, 32 * c4),
                            )
            else:
                for jt in range(JT):
                    for kt in range(KT):
                        nc.tensor.matmul(
                            ps[:, jt * BL:(jt + 1) * BL],
                            W[:, kt * H + jt * P: kt * H + jt * P + P],
                            ringR[:, kt * P + slotR * BL: kt * P + slotR * BL + BL],
                            start=(kt == 0),
                            stop=(kt == KT - 1),
                        )
            # epilogue: h = sign(z) * max(|z| + bm, 0), z = ps + U_t
            z = ep.tile([P, 64], f32, tag=f"{name}z")
            az = ep.tile([P, 64], f32, tag=f"{name}az")
            ab = ep.tile([P, 64], f32, tag=f"{name}ab")
            rr = ep.tile([P, 64], f32, tag=f"{name}rr")
            sg = ep.tile([P, 64], f32, tag=f"{name}sg")
            z3 = z[:].rearrange("p (k b) -> p k b", k=KT)
            nc.vector.tensor_add(z3, ps[:].rearrange("p (k b) -> p k b", k=KT), r3(Usrc, slotU))
            nc.scalar.sign(sg[:], z[:])
            nc.vector.scalar_tensor_tensor(
                az[:], z[:], -1.0, z[:], mybir.AluOpType.mult, mybir.AluOpType.max)
            nc.vector.tensor_add(ab[:], az[:], BMR[:])
            nc.vector.tensor_scalar_max(rr[:], ab[:], 0.0)
            nc.vector.tensor_mul(
                r3(ringW, slotW),
                rr[:].rearrange("p (k b) -> p k b", k=KT),
                sg[:].rearrange("p (k b) -> p k b", k=KT),
            )
            if s == 15:
                flushes.append(nc.sync.dma_start(OUT_dram[:, :, ds(i, 128)], rflush(ringA)).ins)
            elif s == 31:
                flushes.append(nc.sync.dma_start(OUT_dram[:, :, ds(i + 128, 128)], rflush(ringB)).ins)
        # prefetch first half of next body (reads pad junk on the last iter)
        dep_on_producers(nc.sync.dma_start(rflush(UA), U_dram[:, :, ds(i + 256, 128)]))

    if fin_dram is not None:
        nc.sync.dma_start(fin_dram[:], r3(ringB, 15))
    return flushes


def build_nc(T=T_FULL):
    """Build + compile the per-core program. Same SPMD program on all cores."""
    import concourse.tile as tile
    from concourse import bacc

    f32 = _f32()
    nbt = T * BL
    nc = bacc.Bacc("TRN2", target_bir_lowering=False, debug=False, num_devices=NCORES)

    XT = nc.dram_tensor("xt", [P, KT, nbt], f32, kind="ExternalInput").ap()
    WI0 = nc.dram_tensor("wi0t", [D, H], f32, kind="ExternalInput").ap()
    WR0 = nc.dram_tensor("wr0t", [H, H], f32, kind="ExternalInput").ap()
    WI1 = nc.dram_tensor("wi1t", [H, H], f32, kind="ExternalInput").ap()
    WR1 = nc.dram_tensor("wr1t", [H, H], f32, kind="ExternalInput").ap()
    BI0 = nc.dram_tensor("bi0c", [P, JT], f32, kind="ExternalInput").ap()
    BI1 = nc.dram_tensor("bi1c", [P, JT], f32, kind="ExternalInput").ap()
    BM0 = nc.dram_tensor("bm0r", [P, 64], f32, kind="ExternalInput").ap()
    BM1 = nc.dram_tensor("bm1r", [P, 64], f32, kind="ExternalInput").ap()

    OUTT = nc.dram_tensor("outt", [P, KT, nbt + UPAD], f32, kind="ExternalOutput").ap()
    FIN0 = nc.dram_tensor("fin0", [P, 64], f32, kind="ExternalOutput").ap()

    with tile.TileContext(nc) as tc, ExitStack() as top:
        # Internal activations live in a DRAM tile pool so that Tile tracks
        # the cross-phase RAW dependencies (raw dram_tensors are not tracked).
        dp = top.enter_context(tc.tile_pool(name="dram", bufs=1, space="DRAM"))
        U0 = dp.tile([P, KT, nbt + UPAD], f32, tag="u0")
        H0T = dp.tile([P, KT, nbt + UPAD], f32, tag="h0t")
        U1 = dp.tile([P, KT, nbt + UPAD], f32, tag="u1")
        U0, H0T, U1 = U0[:], H0T[:], U1[:]
        # Zero the pad columns: the last loop iteration's U prefetch reads
        # them (by design, as junk) — they must at least be initialized.
        zp = top.enter_context(tc.tile_pool(name="zpad", bufs=1))
        zt = zp.tile([P, KT * 128], f32, tag="zpad")
        nc.vector.memset(zt[:], 0.0)
        z3 = zt[:].rearrange("p (k c) -> p k c", k=KT)
        pad_writes = []
        for U in (U0, U1):
            for o in range(0, UPAD, 128):
                pad_writes.append(
                    nc.sync.dma_start(U[:, :, nbt + o:nbt + o + 128], z3).ins)
        with ExitStack() as ctx:
            w0 = _gemm_phase(ctx, tc, "g0", WI0, BI0, XT, U0, nbt)
        with ExitStack() as ctx:
            fl0 = _rnn_phase(ctx, tc, "r0", WR0, BM0, U0, H0T, nbt,
                             fin_dram=FIN0, after=w0 + pad_writes)
        with ExitStack() as ctx:
            w1 = _gemm_phase(ctx, tc, "g1", WI1, BI1, H0T, U1, nbt, after=fl0)
        with ExitStack() as ctx:
            _rnn_phase(ctx, tc, "r1", WR1, BM1, U1, OUTT, nbt, after=w1 + pad_writes)
    nc.compile()
    return nc


def _prep_inputs(x, Wi0, bi0, Wr0, bm0, Wi1, bi1, Wr1, bm1, T):
    """Host-side shard + transpose. Returns per-core input maps."""

    def col(v):  # [1024] -> [128, 8] feature-tile-major per-partition columns
        return np.ascontiguousarray(v.reshape(JT, P).T)

    def rep(v):  # [1024] -> [128, 64] replicated across the 8 batch columns
        return np.ascontiguousarray(np.repeat(v.reshape(JT, P).T, BL, axis=1))

    shared = {
        "wi0t": np.ascontiguousarray(Wi0.T),
        "wr0t": np.ascontiguousarray(Wr0.T),
        "wi1t": np.ascontiguousarray(Wi1.T),
        "wr1t": np.ascontiguousarray(Wr1.T),
        "bi0c": col(bi0),
        "bi1c": col(bi1),
        "bm0r": rep(bm0),
        "bm1r": rep(bm1),
    }
    in_maps = []
    for c in range(NCORES):
        xc = x[c * BL:(c + 1) * BL, :T, :]  # [8, T, 1024]
        # xt[p, kt, t*8+b] = x[b, t, kt*128+p]
        xt = np.ascontiguousarray(
            xc.reshape(BL, T, KT, P).transpose(3, 2, 1, 0).reshape(P, KT, T * BL)
        )
        in_maps.append({"xt": xt, **shared})
    return in_maps


def _decode_outputs(results, T):
    """results: list of {name: array} per core -> (out [B,T,H], final [2,B,H])."""
    nbt = T * BL
    out = np.empty((B, T, H), np.float32)
    fin = np.empty((2, B, H), np.float32)
    for c in range(NCORES):
        outt = results[c]["outt"][:, :, :nbt]  # [128, 8, nbt]
        # out[b, t, jt*128+p] = outt[p, jt, t*8+b]
        a = outt.reshape(P, JT, T, BL).transpose(3, 2, 1, 0)  # [b, t, jt, p]
        out[c * BL:(c + 1) * BL] = a.reshape(BL, T, H)
        fin[1, c * BL:(c + 1) * BL] = a[:, T - 1].reshape(BL, H)
        f0 = results[c]["fin0"]  # [128, 64] cols = kt*8+b
        fin[0, c * BL:(c + 1) * BL] = f0.reshape(P, KT, BL).transpose(2, 1, 0).reshape(BL, H)
    return out, fin


_NC_CACHE = {}


def _get_nc(T):
    if T not in _NC_CACHE:
        _NC_CACHE[T] = build_nc(T)
    return _NC_CACHE[T]


def run_on_hw(inputs, T=T_FULL, trace=False, tmpdir=None):
    from concourse.bass_utils import run_bass_kernel_spmd

    nc = _get_nc(T)
    in_maps = _prep_inputs(T=T, **inputs)
    res = run_bass_kernel_spmd(
        nc, in_maps, list(range(NCORES)), trace=trace, tmpdir=tmpdir
    )
    out, fin = _decode_outputs(res.results, T)
    return (out, fin), res


def kernel(x, Wi0, bi0, Wr0, bm0, Wi1, bi1, Wr1, bm1):
    inputs = dict(
        x=np.asarray(x, np.float32),
        Wi0=np.asarray(Wi0, np.float32), bi0=np.asarray(bi0, np.float32),
        Wr0=np.asarray(Wr0, np.float32), bm0=np.asarray(bm0, np.float32),
        Wi1=np.asarray(Wi1, np.float32), bi1=np.asarray(bi1, np.float32),
        Wr1=np.asarray(Wr1, np.float32), bm1=np.asarray(bm1, np.float32),
    )
    # The NRT device occasionally reports a transient unrecoverable-exec
    # error right after a previous process released it; one retry clears it.
    last = None
    for _ in range(3):
        try:
            (out, fin), _ = run_on_hw(inputs, T=T_FULL, trace=False)
            return out, fin
        except Exception as e:  # noqa: BLE001
            last = e
            import time
            time.sleep(5)
    raise last
